# revision 18
# baseline (speedup 1.0000x reference)
"""BoundaryLoss (EDT-weighted BCE) on 8 Trainium2 NeuronCores — v2.3.

Layout: per core, partitions = 2 channels x 64 D-rows = 128; free dims =
(H=96, W=24 interior + 8 halo). 8 cores = 2 batches x 4 W-quarters. The
host sends channel 0 = target, channel 1 = 1 - target, so both EDT
channels (dist to background / to foreground) run in one op.

Math: with a binary mask every voxel has dist_pos = 0 or dist_neg = 0, so
w = ramp(dist_pos + dist_neg) = ramp(dist_pos) * ramp(dist_neg) with
ramp(a) = clamp(2.5 - 0.5a, 0, 1): the weight factorizes per channel; the
cross-channel product needs only a quadrant-aligned partition-offset copy.

Passes (separable squared EDT, +-4 window; any value >= 25 gives w = 0, so
values >= 25 only need to STAY >= 25 — all bf16 rounding above 256 and all
cross-row scan leaks land in that zone and are exact for w):
  1. W pass: tensor_tensor_scan with state = t*state + t computes the
     exact 1D distance-to-zero of each channel's mask directly (0 at the
     opposite class, +1 per same-class step). Forward scan + backward scan
     (reversed APs), both split into chained H-halves so they start right
     behind the half-loads; min, then square. The scan chains across H
     rows, but the 8 halo columns between rows make every cross-row leak
     >= 5 steps.
  2. D pass: partition shifts d=4..1 via DMA copies of th_d = dW^2 + d^2
     (2 per direction, one per 64-row channel block) into per-direction
     slots; sentinel rows (100, i.e. out-of-volume) preloaded from DRAM
     into rows [60:64)/[124:128) of all slots, with each direction's data
     DMA overwriting the rows it owns. Mins chase the DMA stream.
  3. H pass: free-dim shifted mins, d=1..4 with edge trimming.

BCE: q = t ? p : 1-p (copy_predicated), bce = min(-ln(q + 3.8e-44), 100);
the tiny Ln bias reproduces torch's log clamp at -100 for q = 0.

Finalize per H-half: a = sqrt(g) on all 128 partitions, u = clamp ramp,
upper channel re-aligned to partitions 0:64, then two tensor_tensor_reduce
ops produce den = sum(u0*u1) and num = sum(m*bce) per D-row. Host reduces
the 8 cores' [64, 4] partials in float64: loss = mean_b(num_b/(den_b+1e-5)).
"""

import numpy as np

B, D, H, W = 2, 64, 96, 96
NQ = 4
WI = W // NQ      # 24 interior columns per core
HALO = 4
WE = WI + 2 * HALO  # 32
N_CORES = B * NQ
HH = H // 2

_CACHE = {}


def _build():
    import concourse.bacc as bacc
    import concourse.mybir as mybir
    import concourse.tile as tile

    fp32 = mybir.dt.float32
    bf16 = mybir.dt.bfloat16
    AF = mybir.ActivationFunctionType
    ALU = mybir.AluOpType

    nc = bacc.Bacc("TRN2", target_bir_lowering=False, debug=False)
    t_d = nc.dram_tensor("t", [128, H, WE], bf16, kind="ExternalInput").ap()
    p_d = nc.dram_tensor("p", [64, H, WI], fp32, kind="ExternalInput").ap()
    s_d = nc.dram_tensor("s", [4, 4, H, WI], bf16, kind="ExternalInput").ap()
    m_d = nc.dram_tensor("sm", [128, 2, 128], bf16, kind="ExternalInput").ap()
    o_d = nc.dram_tensor("o", [64, 4], fp32, kind="ExternalOutput").ap()

    with tile.TileContext(nc) as tc:
        with (
            tc.tile_pool(name="mem", bufs=1) as pool,
            tc.tile_pool(name="ps", bufs=2, space="PSUM") as psp,
        ):
            t2 = pool.tile([128, H, WE], bf16)
            sf = pool.tile([128, H, WE], bf16)
            sb = pool.tile([128, H, WE], bf16)
            m1 = pool.tile([128, H, WI], bf16)
            sq = pool.tile([128, H, WI], bf16)
            th = [pool.tile([128, H, WI], bf16, name=f"th{d}") for d in range(1, 5)]
            bsu = pool.tile([128, 4, H, WI], bf16)  # slot d-1 per direction
            bsd = pool.tile([128, 4, H, WI], bf16)
            gd = pool.tile([128, H, WI], bf16)
            gh = pool.tile([128, H, WI], bf16)
            pp = pool.tile([64, H, WI], fp32)
            q0 = pool.tile([64, H, WI], fp32)
            bce = pool.tile([64, H, WI], bf16)
            da = pool.tile([128, H, WI], bf16)
            uu = pool.tile([128, H, WI], bf16)
            mw = pool.tile([64, H, WI], bf16)
            u1t = pool.tile([64, H, WI], bf16)
            mb = pool.tile([64, H, WI], bf16)
            scr = pool.tile([64, H, WI], bf16)
            tm = pool.tile([64, H, WI], mybir.dt.uint8)
            eps = pool.tile([64, 1], fp32)
            os_ = pool.tile([64, 4], fp32)
            sm = pool.tile([128, 2, 128], bf16)

            def tmin(out_ap, a_ap, b_ap):
                nc.vector.tensor_tensor(out_ap, a_ap, b_ap, op=ALU.min)

            nc.vector.memset(eps[:], 3.7835058e-44)

            # loads: t2 upper H-half first (the backward scan starts there),
            # p + sentinels on the Act queue
            HQ = H - H // 4
            nc.sync.dma_start(t2[:, HQ:H], t_d[:, HQ:H])
            nc.sync.dma_start(t2[:, HH:HQ], t_d[:, HH:HQ])
            nc.sync.dma_start(t2[:, 0:HH], t_d[:, 0:HH])
            with tc.tile_wait_until(0.005):
                nc.scalar.dma_start(pp[:], p_d)
            nc.scalar.dma_start(sm[:], m_d)
            nc.scalar.dma_start(bsu[60:64], s_d)
            nc.scalar.dma_start(bsu[124:128], s_d)
            nc.scalar.dma_start(bsd[0:4], s_d)
            nc.scalar.dma_start(bsd[64:68], s_d)
            nc.scalar.activation(os_[:, 0:1], eps[:], AF.Ln)  # pin ln table

            # ---- W pass: distance scans, state = t*state + t ----
            # chained H-halves; scan order matches load arrival
            t2h = [t2[:, 0:HH].opt(), t2[:, HH:H].opt()]
            sfh = [sf[:, 0:HH].opt(), sf[:, HH:H].opt()]
            sbh = [sb[:, 0:HH].opt(), sb[:, HH:H].opt()]
            t2q = [t2[:, HH:HQ].opt(), t2[:, HQ:H].opt()]
            sbq = [sb[:, HH:HQ].opt(), sb[:, HQ:H].opt()]
            nc.vector.tensor_tensor_scan(
                sbq[1][:, ::-1], t2q[1][:, ::-1], t2q[1][:, ::-1], 100.0,
                op0=ALU.mult, op1=ALU.add)
            nc.vector.tensor_tensor_scan(
                sbq[0][:, ::-1], t2q[0][:, ::-1], t2q[0][:, ::-1],
                sb[:, HQ:HQ + 1, 0:1].opt(), op0=ALU.mult, op1=ALU.add)
            nc.vector.tensor_tensor_scan(
                sfh[0], t2h[0], t2h[0], 100.0, op0=ALU.mult, op1=ALU.add)
            nc.vector.tensor_tensor_scan(
                sfh[1], t2h[1], t2h[1], sf[:, HH - 1:HH, WE - 1:WE].opt(),
                op0=ALU.mult, op1=ALU.add)
            nc.vector.tensor_tensor_scan(
                sbh[0][:, ::-1], t2h[0][:, ::-1], t2h[0][:, ::-1],
                sb[:, HH:HH + 1, 0:1].opt(), op0=ALU.mult, op1=ALU.add)
            lo, hi = HALO, HALO + WI
            tmin(m1[:, 0:HH], sf[:, 0:HH, lo:hi], sb[:, 0:HH, lo:hi])
            tmin(m1[:, HH:H], sf[:, HH:H, lo:hi], sb[:, HH:H, lo:hi])
            nc.vector.tensor_tensor(sq[:, 0:HH], m1[:, 0:HH], m1[:, 0:HH],
                                     op=ALU.mult)
            nc.vector.tensor_tensor(sq[:, HH:H], m1[:, HH:H], m1[:, HH:H],
                                     op=ALU.mult)

            # ---- D pass: partition shifts, d descending ----
            nc.vector.tensor_scalar(th[3][:, 0:HH], sq[:, 0:HH], 1.0, 16.0,
                                    op0=ALU.mult, op1=ALU.add)
            nc.vector.tensor_scalar(th[3][:, HH:H], sq[:, HH:H], 1.0, 16.0,
                                    op0=ALU.mult, op1=ALU.add)
            nc.scalar.activation(th[2][:], sq[:], AF.Copy, bias=9.0)
            nc.scalar.activation(th[1][:], sq[:], AF.Copy, bias=4.0)
            for d in (4, 3, 2):
                slot = d - 1
                thd = th[d - 1]
                if d == 4:
                    for (a, b) in ((0, HH), (HH, H)):
                        nc.sync.dma_start(bsu[0:64 - d, slot, a:b],
                                          thd[d:64, a:b])
                        nc.sync.dma_start(bsu[64:128 - d, slot, a:b],
                                          thd[64 + d:128, a:b])
                else:
                    nc.sync.dma_start(bsu[0:64 - d, slot], thd[d:64])
                    nc.sync.dma_start(bsu[64:128 - d, slot], thd[64 + d:128])
                nc.sync.dma_start(bsd[d:64, slot], thd[0:64 - d])
                nc.sync.dma_start(bsd[64 + d:128, slot], thd[64:128 - d])
                if d == 4:
                    tmin(gd[:], sq[:], bsu[:, slot])   # 3-operand start
                else:
                    tmin(gd[:], bsu[:, slot], gd[:])
                tmin(gd[:], bsd[:, slot], gd[:])
            # d=1 via PE: psum = S.T @ sq per (dir, h-half); boundary rows of
            # S are self-copies so the +1-biased Act drain is a no-op there
            FH = HH * WI  # 1152 = 512 + 512 + 128, bank-aligned chunks
            for i, (a, b) in enumerate(((0, HH), (HH, H))):
                for j, dst in enumerate((bsu, bsd)):
                    pt = psp.tile([128, 3 * 512], fp32, name=f"ps{i}{j}",
                                  tag="ps")
                    sqf = sq[:, a:b, :].opt()
                    for c0 in (0, 512, 1024):
                        c1 = min(c0 + 512, FH)
                        nc.tensor.matmul(pt[:, c0:c1], sm[:, j],
                                         sqf[:, c0:c1], start=True, stop=True)
                    nc.scalar.activation(dst[:, 0, a:b, :].opt(),
                                         pt[:, 0:FH], AF.Copy, bias=1.0)
            tmin(gd[:], bsu[:, 0], gd[:])
            tmin(gd[:], bsd[:, 0], gd[:])

            # ---- BCE (independent; fills the D-phase DMA stalls) ----
            nc.gpsimd.tensor_copy(tm[:], t2[0:64, :, lo:hi])
            with tc.tile_wait_until(0.016):
                nc.vector.tensor_scalar(q0[:, 0:HH], pp[:, 0:HH], -1.0, 1.0,
                                        op0=ALU.mult, op1=ALU.add)
                nc.vector.copy_predicated(q0[:, 0:HH], tm[:, 0:HH], pp[:, 0:HH])
                nc.vector.tensor_scalar(q0[:, HH:H], pp[:, HH:H], -1.0, 1.0,
                                        op0=ALU.mult, op1=ALU.add)
                nc.vector.copy_predicated(q0[:, HH:H], tm[:, HH:H], pp[:, HH:H])
            nc.scalar.activation(scr[:], q0[:], AF.Ln, bias=eps[:])
            with tc.tile_wait_until(0.022):
                nc.vector.tensor_scalar(bce[:], scr[:], -1.0, 100.0,
                                        op0=ALU.mult, op1=ALU.min)

            # ---- H pass: free-dim shifts ----
            nc.vector.tensor_scalar(th[0][:], gd[:], 1.0, 1.0,
                                    op0=ALU.mult, op1=ALU.add)
            nc.vector.tensor_scalar(th[3][:], gd[:], 1.0, 16.0,
                                    op0=ALU.mult, op1=ALU.add)
            nc.scalar.activation(th[1][:], gd[:], AF.Copy, bias=4.0)
            nc.scalar.activation(th[2][:], gd[:], AF.Copy, bias=9.0)
            nc.vector.tensor_copy(gh[:, 0:1, :], gd[:, 0:1, :])
            tmin(gh[:, 1:96], gd[:, 1:96], th[0][:, 0:95])
            tmin(gh[:, 0:95], th[0][:, 1:96], gh[:, 0:95])
            tmin(gh[:, 2:96], th[1][:, 0:94], gh[:, 2:96])
            tmin(gh[:, 0:94], th[1][:, 2:96], gh[:, 0:94])
            tmin(gh[:, 3:96], th[2][:, 0:93], gh[:, 3:96])
            tmin(gh[:, 0:93], th[2][:, 3:96], gh[:, 0:93])
            tmin(gh[:, 4:HH], th[3][:, 0:HH - 4], gh[:, 4:HH])
            tmin(gh[:, 0:HH], th[3][:, 4:HH + 4], gh[:, 0:HH])
            tmin(gh[:, HH:96], th[3][:, HH - 4:92], gh[:, HH:96])
            tmin(gh[:, HH:92], th[3][:, HH + 4:96], gh[:, HH:92])

            # ---- finalize, per H-half ----
            for i, (a, b) in enumerate(((0, HH), (HH, H))):
                hs = np.s_[:, a:b, :]
                nc.scalar.activation(da[hs], gh[hs], AF.Sqrt)
                nc.vector.tensor_scalar(uu[hs], da[hs], -0.5, 2.5,
                                        op0=ALU.mult, op1=ALU.add)
                nc.vector.tensor_scalar(uu[hs], uu[hs], 1.0, 0.0,
                                        op0=ALU.min, op1=ALU.max)
                nc.vector.tensor_scalar(u1t[:, a:b, :], uu[64:128, a:b, :],
                                        1.0, 0.0, op0=ALU.mult, op1=ALU.add)
                nc.vector.tensor_tensor(mw[:, a:b, :], uu[0:64, a:b, :],
                                        u1t[:, a:b, :], op=ALU.mult)
                nc.scalar.activation(scr[:, a:b, :], mw[:, a:b, :], AF.Copy,
                                     accum_out=os_[:, 2 * i + 1:2 * i + 2])
                nc.vector.tensor_tensor(mb[:, a:b, :], mw[:, a:b, :],
                                        bce[:, a:b, :], op=ALU.mult)
                if i == 0:
                    nc.scalar.activation(scr[:, a:b, :], mb[:, a:b, :],
                                         AF.Copy,
                                         accum_out=os_[:, 2 * i:2 * i + 1])
                else:
                    nc.vector.tensor_reduce(os_[:, 2 * i:2 * i + 1],
                                            mb[:, a:b, :],
                                            mybir.AxisListType.XY, op=ALU.add)
                nc.sync.dma_start(o_d[:, 2 * i:2 * i + 2],
                                  os_[:, 2 * i:2 * i + 2])
    nc.compile()
    return nc


def _get_nc():
    if "nc" not in _CACHE:
        _CACHE["nc"] = _build()
    return _CACHE["nc"]


def _slabs(pred, target):
    import ml_dtypes

    bf16 = ml_dtypes.bfloat16
    tp = np.pad(
        np.asarray(target, dtype=np.float32),
        ((0, 0), (0, 0), (0, 0), (HALO, HALO)),
        mode="edge",
    )  # [B, D, H, W+8]
    pr = np.asarray(pred, dtype=np.float32)
    sent = np.full((4, 4, H, WI), 100.0, dtype=bf16)
    smat = np.zeros((128, 2, 128), dtype=np.float32)
    for p in range(128):
        up = p + 1 if (p % 64) < 63 else p
        dn = p - 1 if (p % 64) > 0 else p
        smat[up, 0, p] = 1.0
        smat[dn, 1, p] = 1.0
    smat = smat.astype(bf16)
    in_maps = []
    for b in range(B):
        for q in range(NQ):
            ts_ = np.ascontiguousarray(
                tp[b, :, :, q * WI: q * WI + WE]
            )  # [64, H, WE]
            t2 = np.concatenate([ts_, 1.0 - ts_], axis=0).astype(bf16)
            ps = np.ascontiguousarray(pr[b, :, :, q * WI:(q + 1) * WI])
            in_maps.append({"t": t2, "p": ps, "s": sent, "sm": smat})
    return in_maps


def kernel(pred: np.ndarray, target: np.ndarray) -> np.ndarray:
    from concourse.bass_utils import run_bass_kernel_spmd

    nc = _get_nc()
    in_maps = _slabs(pred, target)
    res = run_bass_kernel_spmd(nc, in_maps, list(range(N_CORES)))

    loss = 0.0
    for b in range(B):
        num = 0.0
        den = 0.0
        for q in range(NQ):
            o = res.results[b * NQ + q]["o"].astype(np.float64)
            num += o[:, 0].sum() + o[:, 2].sum()
            den += o[:, 1].sum() + o[:, 3].sum()
        loss += num / (den + 1e-5)
    return np.float32(loss / B)
